# revision 6
# baseline (speedup 1.0000x reference)
"""Trainium2 Bass kernel for 3-layer GCN + Linear + log_softmax (v2).

The baseline spent its whole 33.8ms in DMAGatherAnt descriptor generation
on the single Q7 (gpsimd) engine (~70 cycles/edge-descriptor).  v2 uses the
`ap_gather` SBUF->SBUF ucode instead: measured 27.1ns per index per core,
with the 8 Q7 cores running 8 independent index streams in parallel and
each index moving a full 32-feature edge message (bf16 d=2 feature-pair
rows, 16 partitions x 2 features) => ~3.4 ns/edge.

Layout: feature-major, nodes sharded 8 ways.  Core-chunk k (partitions
[16k,16k+16)) holds the full u table for shard k: table[16k+p, 2n+a] =
feature (16a+p) of shard-k node n, bf16.  Per layer:

  GEMM (quadrant-split W, K=16 accumulate) on dinv-prescaled activations
  -> u chunks -> DRAM shard -> AllGather -> table (straight DMA, the
  AllGather concat IS the table layout) -> NCALLS ap_gather calls of C
  idxs/core (per-core edge streams bucketed by src shard, dst segments
  rank-sorted by per-chunk in-degree; uniform segment length P per
  (chunk-pair, call)) -> strided tensor_reduce into bf16 partial[rank]
  -> unpermute (ap_gather d=2 over partial) -> masked add of the
  self-loop u_own term -> stacked-identity PE matmuls sum the 8 chunks
  -> z = dinv*S, x' = relu(z+b), prescale by dinv for the next GEMM.

Final layer: logits feature-major; log_softmax = logits - ones^T @
ln(mask6^T @ exp(logits)) via two tiny matmuls.  Host de-shards.
"""

import math

import numpy as np

from concourse import bass, mybir, bacc, tile
from concourse.bass_utils import run_bass_kernel_spmd

F32 = mybir.dt.float32
F16 = mybir.dt.float16
BF16 = mybir.dt.bfloat16
I16 = mybir.dt.int16

N_CORES = 8
PER = 12500
SLOTS = 12544
NP = 4  # chunk pairs
DP = 32
DIMS = (16, 32, 24, 12, 6)
C = 2048  # gather-call indices per core
UCH = 2048  # unpermute chunk columns
ECH = 512  # epilogue / GEMM chunk
PADIDX = SLOTS - 1  # dummy node slot (always-zero table column)

LAST_RUN_INFO = {}


def _bf16(a):
    import ml_dtypes

    return np.asarray(a).astype(ml_dtypes.bfloat16)


def _f16(a):
    return np.asarray(a).astype(np.float16)


class _Plan:
    pass


# ---------------------------------------------------------------------------
# Host-side plan (pure integer index manipulation)
# ---------------------------------------------------------------------------
def _build_plan(edge_index, n_nodes):
    src = np.asarray(edge_index[0], dtype=np.int64)
    dst = np.asarray(edge_index[1], dtype=np.int64)
    assert n_nodes == N_CORES * PER

    dst_core = dst // PER
    dst_slot = dst % PER
    src_chunk = src // PER
    src_slot = src % PER

    # per (core, chunk, slot) in-degree
    deg = np.zeros((N_CORES, N_CORES, SLOTS), dtype=np.int64)
    np.add.at(deg, (dst_core, src_chunk, dst_slot), 1)

    pi = np.argsort(-deg, axis=2, kind="stable")  # [core, chunk, SLOTS]
    deg_sorted = np.take_along_axis(deg, pi, axis=2)
    # shared P per (chunk-pair, rank): max over cores and the pair
    P_rank = deg_sorted.reshape(N_CORES, NP, 2, SLOTS).max(axis=(0, 2))  # [NP, SLOTS]

    calls = []  # per pair: list of (rank_start, n, P)
    for q in range(NP):
        ca = []
        r = 0
        while r < SLOTS and P_rank[q, r] > 0:
            P = int(P_rank[q, r])
            n = min(C // P, SLOTS - r)
            ca.append((r, n, P))
            r += n
        calls.append(ca)
    NCALLS = max(len(ca) for ca in calls)
    infl = sum(n * P for ca in calls for (_, n, P) in ca) * N_CORES * 2 / max(len(src), 1)
    print(f"plan: NCALLS={NCALLS} pad inflation={infl:.3f}")

    # sort edges by (dst_core, src_chunk, dst_slot) for segment extraction
    order = np.lexsort((dst_slot, src_chunk, dst_core))
    s_core = dst_core[order]
    s_chunk = src_chunk[order]
    s_slot = dst_slot[order]
    s_src = src_slot[order].astype(np.int16)
    key = (s_core * N_CORES + s_chunk) * SLOTS + s_slot
    seg_start = np.searchsorted(key, np.arange(N_CORES * N_CORES * SLOTS + 1))

    idx16 = np.full((N_CORES, 128, NCALLS * (C // 16)), PADIDX, dtype=np.int16)
    for cidx in range(N_CORES):
        for k in range(N_CORES):
            q = k // 2
            stream = np.full(NCALLS * C, PADIDX, dtype=np.int16)
            for ci, (r0, n, P) in enumerate(calls[q]):
                base = ci * C
                ranks = pi[cidx, k, r0 : r0 + n]
                degs = deg[cidx, k, ranks]
                kk = (cidx * N_CORES + k) * SLOTS + ranks
                st = seg_start[kk]
                for j in range(n):
                    d = degs[j]
                    if d:
                        stream[base + j * P : base + j * P + d] = s_src[
                            st[j] : st[j] + d
                        ]
            idx16[cidx, 16 * k : 16 * k + 16, :] = stream.reshape(-1, 16).T

    up16 = np.zeros((N_CORES, 128, SLOTS // 16), dtype=np.int16)
    for cidx in range(N_CORES):
        for k in range(N_CORES):
            inv = np.empty(SLOTS, dtype=np.int64)
            inv[pi[cidx, k]] = np.arange(SLOTS)
            up16[cidx, 16 * k : 16 * k + 16, :] = (
                inv.astype(np.int16).reshape(-1, 16).T
            )

    deg_tot = deg.sum(axis=1) + 1  # self loop
    deg_tot[:, PER:] = 1

    pl = _Plan()
    pl.calls = calls
    pl.NCALLS = NCALLS
    pl.idx16 = idx16
    pl.up16 = up16
    pl.deg_tot = deg_tot
    return pl


def _make_in_maps(pl, x, W1, b1, W2, b2, W3, b3, Wf, bf):
    x = np.asarray(x, dtype=np.float32)
    Ws = [np.asarray(W, np.float32) for W in (W1, W2, W3, Wf)]
    bs = [np.asarray(b, np.float32) for b in (b1, b2, b3, bf)]
    # W quadrants: Wq[l, fi_half, fo_half] = W[16*fi:+16, 16*fo:+16]
    Wq = np.zeros((4, 2, 2, 16, 16), dtype=np.float32)
    bq = np.zeros((4, 2, 16, 1), dtype=np.float32)
    for l, (W, b) in enumerate(zip(Ws, bs)):
        Wp = np.zeros((DP, DP), np.float32)
        Wp[: W.shape[0], : W.shape[1]] = W
        for a in range(2):
            for o in range(2):
                Wq[l, a, o] = Wp[16 * a : 16 * a + 16, 16 * o : 16 * o + 16]
        bp = np.zeros(DP, np.float32)
        bp[: b.shape[0]] = b
        bq[l, 0, :, 0] = bp[:16]
        bq[l, 1, :, 0] = bp[16:]
    W_flat = _bf16(Wq.transpose(3, 0, 1, 2, 4).reshape(16, 4 * 2 * 2 * 16))
    b_flat = bq.transpose(2, 0, 1, 3).reshape(16, 8)

    stack8 = np.zeros((128, 16), dtype=np.float32)
    for r in range(128):
        stack8[r, r % 16] = 1.0
    mask6 = np.zeros((16, 1), dtype=np.float32)
    mask6[:6, 0] = 1.0
    ones16 = np.ones((1, 16), dtype=np.float32)

    in_maps = []
    for c in range(N_CORES):
        xT = np.zeros((DP, SLOTS), dtype=np.float32)
        xT[: x.shape[1], :PER] = x[c * PER : (c + 1) * PER].T
        mask = np.zeros((128, 1), dtype=np.float32)
        mask[16 * c : 16 * c + 16, 0] = 1.0
        in_maps.append(
            {
                "xlo_in": _bf16(xT[:16]),
                "xhi_in": _bf16(xT[16:]),
                "idx_in": np.ascontiguousarray(pl.idx16[c]),
                "up_in": np.ascontiguousarray(pl.up16[c]),
                "deg_in": _f16(
                    np.broadcast_to(
                        pl.deg_tot[c].reshape(8, 1, SLOTS // 8), (8, 16, SLOTS // 8)
                    ).reshape(128, SLOTS // 8)
                ),
                "W_in": W_flat,
                "b_in": b_flat,
                "s8_in": _bf16(stack8),
                "m6_in": mask6,
                "o16_in": ones16,
                "mask_in": _bf16(mask),
            }
        )
    return in_maps


def _assemble_output(outs_per_core):
    full = np.empty((N_CORES * PER, 6), dtype=np.float32)
    for c in range(N_CORES):
        full[c * PER : (c + 1) * PER] = outs_per_core[c][:, :PER].T
    return full


# ---------------------------------------------------------------------------
# Device kernel
# ---------------------------------------------------------------------------
def _build_kernel(pl):
    NCALLS = pl.NCALLS
    T16 = NCALLS * (C // 16)
    U16 = SLOTS // 16

    nc = bacc.Bacc("TRN2", target_bir_lowering=False, debug=False, num_devices=N_CORES)

    xlo_in = nc.dram_tensor("xlo_in", [16, SLOTS], BF16, kind="ExternalInput")
    xhi_in = nc.dram_tensor("xhi_in", [16, SLOTS], BF16, kind="ExternalInput")
    idx_in = nc.dram_tensor("idx_in", [128, T16], I16, kind="ExternalInput")
    up_in = nc.dram_tensor("up_in", [128, U16], I16, kind="ExternalInput")
    deg_in = nc.dram_tensor("deg_in", [128, SLOTS // 8], F16, kind="ExternalInput")
    W_in = nc.dram_tensor("W_in", [16, 256], BF16, kind="ExternalInput")
    b_in = nc.dram_tensor("b_in", [16, 8], F32, kind="ExternalInput")
    s8_in = nc.dram_tensor("s8_in", [128, 16], BF16, kind="ExternalInput")
    m6_in = nc.dram_tensor("m6_in", [16, 1], F32, kind="ExternalInput")
    o16_in = nc.dram_tensor("o16_in", [1, 16], F32, kind="ExternalInput")
    mask_in = nc.dram_tensor("mask_in", [128, 1], BF16, kind="ExternalInput")
    out_dram = nc.dram_tensor("out", [6, SLOTS], F32, kind="ExternalOutput")

    SPL = 9216
    shardA = {}
    shardB = {}
    tabdA = {}
    tabdB = {}
    for k in (1, 2, 3):
        shardA[k] = nc.dram_tensor(f"shardA{k}", [16, 2 * SPL], BF16)
        shardB[k] = nc.dram_tensor(f"shardB{k}", [16, 2 * (SLOTS - SPL)], BF16)
        tabdA[k] = nc.dram_tensor(
            f"tableA{k}", [128, 2 * SPL], BF16, addr_space="Shared"
        )
        tabdB[k] = nc.dram_tensor(
            f"tableB{k}", [128, 2 * (SLOTS - SPL)], BF16, addr_space="Shared"
        )
    rgroups = [list(range(N_CORES))]

    echunks = []
    o = 0
    while o < SLOTS:
        w = min(ECH, SLOTS - o)
        echunks.append((o, w))
        o += w
    uchunks = []
    o = 0
    while o < SLOTS:
        w = min(UCH, SLOTS - o)
        uchunks.append((o, w))
        o += w

    with tile.TileContext(nc, num_cores=N_CORES) as tc:
        with (
            tc.tile_pool(name="pers", bufs=1) as pers,
            tc.tile_pool(name="msg", bufs=2) as mpool,
            tc.tile_pool(name="und", bufs=2) as upool,
            tc.tile_pool(name="idxp", bufs=3) as ipool,
            tc.tile_pool(name="chunk", bufs=4) as spool,
            tc.tile_pool(name="row", bufs=1) as rpool,
            tc.tile_pool(name="ps", bufs=6, space="PSUM") as ppool,
            tc.tile_pool(name="ps1", bufs=2, space="PSUM") as ppool1,
        ):
            table = pers.tile([128, 2 * SLOTS], BF16)
            partial = pers.tile([128, 2 * SLOTS], BF16)
            xlo = pers.tile([16, SLOTS], BF16)
            xhi = pers.tile([16, SLOTS], BF16)
            dinvb = pers.tile([16, SLOTS], F16)
            W_sb = pers.tile([16, 256], BF16)
            b_sb = pers.tile([16, 8], F32)
            s8 = pers.tile([128, 16], BF16)
            m6 = pers.tile([16, 1], F32)
            o16 = pers.tile([1, 16], F32)
            maskt = pers.tile([128, 1], BF16)

            nc.sync.dma_start(W_sb[:], W_in[:, :])
            nc.sync.dma_start(b_sb[:], b_in[:, :])
            nc.sync.dma_start(s8[:], s8_in[:, :])
            nc.sync.dma_start(m6[:], m6_in[:, :])
            nc.sync.dma_start(o16[:], o16_in[:, :])
            nc.sync.dma_start(maskt[:], mask_in[:, :])
            dtmp = upool.tile([128, SLOTS // 8], F16, name="dtmp", tag="und")
            nc.sync.dma_start(dtmp[:], deg_in[:, :])
            with nc.allow_low_precision(reason="f16 dinv"):
                nc.vector.reciprocal(dtmp[:], dtmp[:])
            nc.scalar.activation(
                out=dtmp[:], in_=dtmp[:], func=mybir.ActivationFunctionType.Sqrt
            )
            for cc in range(8):
                J8 = SLOTS // 8
                nc.sync.dma_start(
                    dinvb[:, cc * J8 : (cc + 1) * J8],
                    dtmp[16 * cc : 16 * cc + 16, :],
                )
            nc.sync.dma_start(xlo[:], xlo_in[:, :])
            nc.sync.dma_start(xhi[:], xhi_in[:, :])

            def Wquad(l, a, o):
                col = ((l * 2 + a) * 2 + o) * 16
                return W_sb[:, col : col + 16]

            def bvec(l, h):
                return b_sb[:, l * 2 + h : l * 2 + h + 1]

            def emit_gemm(kk, o, w):
                # u_kk = (dinv*x_kk) @ W_kk for columns [o, o+w) -> shard[kk]
                l = kk - 1
                uch = spool.tile([16, 2 * ECH], BF16, name=f"u{kk}_{o}", tag="chunk")
                for h in (0, 1):
                    psg = ppool.tile(
                        [16, ECH], F32, space="PSUM", name=f"g{kk}_{o}_{h}", tag="ps"
                    )
                    nc.tensor.matmul(
                        psg[:, :w], lhsT=Wquad(l, 0, h), rhs=xlo[:, o : o + w],
                        start=True, stop=False,
                    )
                    nc.tensor.matmul(
                        psg[:, :w], lhsT=Wquad(l, 1, h), rhs=xhi[:, o : o + w],
                        start=False, stop=True,
                    )
                    # cast + interleave write: u_chunk[p, 2j+h] = psg[p, j]
                    nc.vector.tensor_copy(
                        uch[:, h : 2 * w : 2].rearrange("p (w one) -> p w one", one=1),
                        psg[:, :w],
                    )
                if o < SPL:
                    nc.sync.dma_start(
                        shardA[kk][:, 2 * o : 2 * (o + w)], uch[:, : 2 * w]
                    )
                else:
                    nc.sync.dma_start(
                        shardB[kk][:, 2 * (o - SPL) : 2 * (o - SPL + w)],
                        uch[:, : 2 * w],
                    )
                if o + w == SPL:
                    nc.gpsimd.collective_compute(
                        "AllGather",
                        mybir.AluOpType.bypass,
                        replica_groups=rgroups,
                        ins=[shardA[kk].ap().opt()],
                        outs=[tabdA[kk].ap().opt()],
                    )
                elif o + w == SLOTS:
                    nc.gpsimd.collective_compute(
                        "AllGather",
                        mybir.AluOpType.bypass,
                        replica_groups=rgroups,
                        ins=[shardB[kk].ap().opt()],
                        outs=[tabdB[kk].ap().opt()],
                    )

            def emit_final(o, w):
                # logits + log_softmax for columns [o, o+w) -> out_dram
                psg = ppool.tile([16, ECH], F32, space="PSUM", name=f"gf_{o}", tag="ps")
                nc.tensor.matmul(
                    psg[:, :w], lhsT=Wquad(3, 0, 0), rhs=xlo[:, o : o + w],
                    start=True, stop=False,
                )
                nc.tensor.matmul(
                    psg[:, :w], lhsT=Wquad(3, 1, 0), rhs=xhi[:, o : o + w],
                    start=False, stop=True,
                )
                logits = spool.tile([16, ECH], F32, name=f"lg_{o}", tag="chunk")
                nc.vector.tensor_scalar_add(logits[:, :w], psg[:, :w], bvec(3, 0))
                ez = spool.tile([16, ECH], F32, name=f"e_{o}", tag="chunk")
                nc.scalar.activation(
                    out=ez[:, :w], in_=psg[:, :w],
                    func=mybir.ActivationFunctionType.Exp,
                    bias=bvec(3, 0), scale=1.0,
                )
                pss = ppool1.tile([1, ECH], F32, space="PSUM", name=f"sf_{o}", tag="ps1")
                nc.tensor.matmul(
                    pss[:, :w], lhsT=m6[:], rhs=ez[:, :w], start=True, stop=True
                )
                lnr = rpool.tile([1, ECH], F32, name=f"ln_{o}", tag="row")
                nc.scalar.activation(
                    out=lnr[:, :w], in_=pss[:, :w],
                    func=mybir.ActivationFunctionType.Ln,
                )
                psb = ppool.tile([16, ECH], F32, space="PSUM", name=f"bc_{o}", tag="ps")
                nc.tensor.matmul(
                    psb[:, :w], lhsT=o16[:], rhs=lnr[:, :w], start=True, stop=True
                )
                oc = spool.tile([16, ECH], F32, name=f"oc_{o}", tag="chunk")
                nc.vector.tensor_sub(oc[:, :w], logits[:, :w], psb[:, :w])
                nc.sync.dma_start(out_dram[:, o : o + w], oc[0:6, :w])

            # chunked dinv = rsqrt(deg) + prescale + layer-1 GEMM pipeline:
            # the first GEMM starts as soon as the first 512 columns of dinv
            # are ready instead of waiting ~90us for the whole tile
            for o, w in echunks:
                for xh in (xlo, xhi):
                    nc.vector.tensor_tensor(
                        out=xh[:, o : o + w], in0=xh[:, o : o + w],
                        in1=dinvb[:, o : o + w], op=mybir.AluOpType.mult,
                    )
                emit_gemm(1, o, w)

            for k in (1, 2, 3):
                l = k - 1
                nc.sync.dma_start(table[:, : 2 * SPL], tabdA[k].ap())
                nc.sync.dma_start(table[:, 2 * SPL :], tabdB[k].ap())
                nc.vector.memset(partial[:], 0.0)

                # ---- gather + segment reduce ----
                for i in range(NCALLS):
                    it = ipool.tile([128, C // 16], I16, name=f"it{k}_{i}", tag="idx")
                    nc.sync.dma_start(
                        it[:], idx_in[:, i * (C // 16) : (i + 1) * (C // 16)]
                    )
                    msg = mpool.tile([128, 2 * C], BF16, name=f"m{k}_{i}", tag="msg")
                    nc.gpsimd.ap_gather(
                        out_ap=msg[:],
                        in_ap=table[:],
                        idxs_ap=it[:],
                        channels=128,
                        num_elems=SLOTS,
                        d=2,
                        num_idxs=C,
                    )
                    with nc.allow_low_precision(reason="bf16 partials"):
                        for q in range(NP):
                            if i >= len(pl.calls[q]):
                                continue
                            r0, n, P = pl.calls[q][i]
                            nc.vector.tensor_reduce(
                                out=partial[32 * q : 32 * q + 32, 2 * r0 : 2 * (r0 + n)],
                                in_=msg[32 * q : 32 * q + 32, : 2 * n * P].rearrange(
                                    "p (n P two) -> p n two P", P=P, two=2
                                ),
                                axis=mybir.AxisListType.X,
                                op=mybir.AluOpType.add,
                            )

                # ---- unpermute + self-term + chunk-sum + epilogue ----
                for uo, uw in uchunks:
                    iu = ipool.tile([128, UCH // 16], I16, name=f"iu{k}_{uo}", tag="idx")
                    nc.sync.dma_start(
                        iu[:, : uw // 16], up_in[:, uo // 16 : (uo + uw) // 16]
                    )
                    unp = mpool.tile([128, 2 * UCH], BF16, name=f"up{k}_{uo}", tag="msg")
                    nc.gpsimd.ap_gather(
                        out_ap=unp[:, : 2 * uw],
                        in_ap=partial[:],
                        idxs_ap=iu[:, : uw // 16],
                        channels=128,
                        num_elems=SLOTS,
                        d=2,
                        num_idxs=uw,
                    )
                    # unp += mask * u_own (self-loop term, natural order)
                    nc.vector.scalar_tensor_tensor(
                        out=unp[:, : 2 * uw],
                        in0=table[:, 2 * uo : 2 * (uo + uw)],
                        scalar=maskt[:],
                        in1=unp[:, : 2 * uw],
                        op0=mybir.AluOpType.mult,
                        op1=mybir.AluOpType.add,
                    )
                    # deinterleave halves
                    ua = upool.tile([128, UCH], BF16, name=f"ua{k}_{uo}", tag="und")
                    ub = upool.tile([128, UCH], BF16, name=f"ub{k}_{uo}", tag="und")
                    nc.vector.tensor_copy(
                        ua[:, :uw],
                        unp[:, : 2 * uw].rearrange("p (w two) -> p two w", two=2)[
                            :, 0:1, :
                        ],
                    )
                    nc.vector.tensor_copy(
                        ub[:, :uw],
                        unp[:, : 2 * uw].rearrange("p (w two) -> p two w", two=2)[
                            :, 1:2, :
                        ],
                    )
                    for so in range(0, uw, ECH):
                        w = min(ECH, uw - so)
                        o = uo + so
                        for h, uh in ((0, ua), (1, ub)):
                            pss = ppool.tile(
                                [16, ECH], F32, space="PSUM",
                                name=f"s{k}_{o}_{h}", tag="ps",
                            )
                            nc.tensor.matmul(
                                pss[:, :w], lhsT=s8[:], rhs=uh[:, so : so + w],
                                start=True, stop=True,
                            )
                            zc = spool.tile(
                                [16, ECH], BF16, name=f"z{k}_{o}_{h}", tag="chunk"
                            )
                            nc.vector.tensor_copy(zc[:, :w], pss[:, :w])
                            nc.vector.tensor_tensor(
                                out=zc[:, :w], in0=zc[:, :w],
                                in1=dinvb[:, o : o + w], op=mybir.AluOpType.mult,
                            )
                            xh = xlo if h == 0 else xhi
                            nc.scalar.activation(
                                out=xh[:, o : o + w], in_=zc[:, :w],
                                func=mybir.ActivationFunctionType.Relu,
                                bias=bvec(l, h), scale=1.0,
                            )
                            if k < 3:
                                nc.vector.tensor_tensor(
                                    out=xh[:, o : o + w], in0=xh[:, o : o + w],
                                    in1=dinvb[:, o : o + w], op=mybir.AluOpType.mult,
                                )
                        # fused next-stage for these freshly-written columns:
                        # hides under the remaining unperm gather calls
                        if k < 3:
                            if o + w > PER:
                                nc.vector.memset(xlo[:, PER:SLOTS], 0.0)
                                nc.vector.memset(xhi[:, PER:SLOTS], 0.0)
                            emit_gemm(k + 1, o, w)
                        else:
                            emit_final(o, w)

    nc.compile()
    return nc


# ---------------------------------------------------------------------------
# Entry point
# ---------------------------------------------------------------------------
def kernel(x, edge_index, W1, b1, W2, b2, W3, b3, Wf, bf):
    x = np.asarray(x, dtype=np.float32)
    pl = _build_plan(np.asarray(edge_index), x.shape[0])
    nc = _build_kernel(pl)
    in_maps = _make_in_maps(pl, x, W1, b1, W2, b2, W3, b3, Wf, bf)

    res = run_bass_kernel_spmd(nc, in_maps, core_ids=list(range(N_CORES)))

    LAST_RUN_INFO.clear()
    LAST_RUN_INFO["exec_time_ns"] = res.exec_time_ns
    LAST_RUN_INFO["mean_exec_time_ns"] = res.mean_exec_time_ns

    outs = [res.results[c]["out"] for c in range(N_CORES)]
    return _assemble_output(outs)
